# revision 3
# baseline (speedup 1.0000x reference)
"""MetaSR super-resolution Trainium2 kernel (bf16, single-copy band).

Structure exploited: out_h=out_w=256 with H=W=64 LR grid means the scale
factor is exactly 4, so the nearest-neighbor gather index is iy=oy//4,
ix=ox//4 and the per-query MLP input collapses to 16 distinct subpixel
phases [dy/4, dx/4, 0.25].  The whole model becomes:

  1. h    = relu(mlp_in @ w1 + b1)              [16, 256]
  2. predw = h @ w2 + b2                        [16, 576, 3]
  3. rgb[o, 4*iy+dy, 4*ix+dx] =
       sum_{c,ki,kj} feat[c, iy+ki-1, ix+kj-1] * predw[(dy,dx), c*9+ki*3+kj, o]
     i.e. a 3x3 conv with 64 in / 48 out channels + pixel shuffle.

Sharding: data-parallel over LR rows (8 rows per core, 10-row halo band),
weights replicated; steps 1+2 are recomputed on every core (tiny).

The conv contraction runs as 9 K=64 matmuls (one per 3x3 tap) against a
single zero-padded band tile [64, 662]; tap (ki,kj) is a free-dim offset
ki*66+kj into the 66-wide padded rows, so the unfolded tensor is never
materialized and the band is DMA'd exactly once (84KB).

All large operands are bf16 (host-side cast); verified end-to-end rel err
~4e-3 vs the f32 reference (budget 2e-2).

Scheduling: one DMA queue (Sync) carries every input blob in strict
consumption order (w1+bias first at 84KB, then per-tap w2 chunks with the
band between taps 1 and 2); the output rides the otherwise-idle Scalar
queue.  The PE is kept continuously busy with dummy "warm"/"filler"
matmuls wherever the instruction stream would otherwise stall on DMA --
the TRN2 PE clock drops from 2.4GHz to 1.2GHz whenever the engine goes
idle and takes ~3us of continuous work to ramp back.
"""

import os

import numpy as np

try:
    import concourse.bass as bass
except ImportError:  # fall back to the repo checkout
    import sys
    sys.path.insert(0, "/opt/trn_rl_repo")
    import concourse.bass as bass
import concourse.mybir as mybir
import concourse.tile as tile
from concourse import bacc
from concourse.bass_utils import run_bass_kernel_spmd

F32 = mybir.dt.float32
BF16 = mybir.dt.bfloat16
N_CORES = 8
ROWS_PER_CORE = 8          # LR rows per core
BAND_ROWS = ROWS_PER_CORE + 2
NPOS = ROWS_PER_CORE * 64  # 512 LR positions per core

# blob_w1x [128, 330] bf16: bias (29 f32 = 58 bf16 cols) | w1 [3,256] | mlpin [3,16]
OFF_W1 = 58
OFF_MLP = 58 + 256
COLS_W1X = 330
# per-tap w2 blocks: (o*2+hc) x [128, 64] = 384 cols each
# blob_a: taps 0-1 | blob_b: taps 2-5 | blob_c: taps 6-8
COLS_A = 2 * 384
COLS_B = 4 * 384
COLS_C = 3 * 384
COLS_BAND = 662

def _env(name, default):
    return int(os.environ.get(name, str(default)))

WARM_BIG = _env("METASR_WARM_BIG", 2)
WARM_SMALL = _env("METASR_WARM_SMALL", 2)
FILL_MLP = _env("METASR_FILL_MLP", 3)     # after mlp1, waiting blob_a
FILL_BAND = _env("METASR_FILL_BAND", 2)   # before conv t0, waiting band
FILL_B = _env("METASR_FILL_B", 4)         # before asm t2, waiting blob_b
FILL_C = _env("METASR_FILL_C", 2)         # before asm t6, waiting blob_c

_CACHE = {}


def _build_program(cfg):
    """Build + compile the single-core Bass program (same for all cores)."""
    warm_big, warm_small, fill_mlp, fill_band, fill_b, fill_c = cfg
    nc = bacc.Bacc("TRN2", target_bir_lowering=False, debug=False)

    blob_w1x_d = nc.dram_tensor("blob_w1x", [128, COLS_W1X], BF16, kind="ExternalInput")
    blob_a_d = nc.dram_tensor("blob_a", [128, COLS_A], BF16, kind="ExternalInput")
    blob_band_d = nc.dram_tensor("blob_band", [64, COLS_BAND], BF16, kind="ExternalInput")
    blob_b_d = nc.dram_tensor("blob_b", [128, COLS_B], BF16, kind="ExternalInput")
    blob_c_d = nc.dram_tensor("blob_c", [128, COLS_C], BF16, kind="ExternalInput")
    out48 = nc.dram_tensor("out48", [48, NPOS], BF16, kind="ExternalOutput")

    with tile.TileContext(nc) as tc:
        with (
            tc.tile_pool(name="blobs", bufs=1) as blobs,
            tc.tile_pool(name="work", bufs=1) as work,
            tc.tile_pool(name="wpool", bufs=5) as wpool,
            tc.tile_pool(name="opool", bufs=1) as opool,
            tc.tile_pool(name="ps_small", bufs=2, space="PSUM") as ps_small,
            tc.tile_pool(name="ps_w", bufs=4, space="PSUM") as ps_w,
            tc.tile_pool(name="ps_warm", bufs=1, space="PSUM") as ps_warm,
            tc.tile_pool(name="ps_rgb", bufs=1, space="PSUM") as ps_rgb,
        ):
            # One queue (Sync), strict consumption order; out on Scalar.
            blob_w1x = blobs.tile([128, COLS_W1X], BF16, tag="blob_w1x")
            nc.sync.dma_start(blob_w1x[:, :], blob_w1x_d[:, :])
            blob_a = blobs.tile([128, COLS_A], BF16, tag="blob_a")
            nc.sync.dma_start(blob_a[:, :], blob_a_d[:, :])
            blob_band = blobs.tile([64, COLS_BAND], BF16, tag="blob_band")
            nc.sync.dma_start(blob_band[:, :], blob_band_d[:, :])
            blob_b = blobs.tile([128, COLS_B], BF16, tag="blob_b")
            nc.sync.dma_start(blob_b[:, :], blob_b_d[:, :])
            blob_c = blobs.tile([128, COLS_C], BF16, tag="blob_c")
            nc.sync.dma_start(blob_c[:, :], blob_c_d[:, :])

            w1_sb = blob_w1x[0:3, OFF_W1:OFF_W1 + 256]
            mlp_sb = blob_w1x[0:3, OFF_MLP:OFF_MLP + 16]
            bias = blob_w1x.bitcast(F32)[:, 0:29]

            def w2_slice(t, o, hc):
                col = (o * 2 + hc) * 64
                if t < 2:
                    return blob_a[:, t * 384 + col:t * 384 + col + 64]
                if t < 6:
                    return blob_b[:, (t - 2) * 384 + col:(t - 2) * 384 + col + 64]
                return blob_c[:, (t - 6) * 384 + col:(t - 6) * 384 + col + 64]

            # ---- PE warm-up / fillers: dummy zero matmuls keep the clock
            # ramped; they write scratch PSUM (or rgb_ps before conv t0,
            # which resets accumulation with start=True).
            rgb_ps = ps_rgb.tile([48, NPOS], F32, tag="rgb")
            warm_ps = ps_warm.tile([128, 128], F32, tag="warm_ps")
            warm = work.tile([128, 512], BF16, tag="warm")
            nc.vector.memset(warm[:, :], 0.0)

            def filler(n):
                for _ in range(n):
                    nc.tensor.matmul(
                        warm_ps[:, :], warm[:, 0:128], warm[:, 0:128],
                        start=True, stop=True,
                    )

            for _ in range(warm_big):
                nc.tensor.matmul(
                    rgb_ps[:, :], warm[:, 0:48], warm[:, 0:NPOS],
                    start=True, stop=True,
                )
            filler(warm_small)

            # ---- MLP layer 1: h_actT [256, 16] in two 128-chunks ----
            h_sb = work.tile([128, 32], BF16, tag="hact")
            for hc in range(2):
                ph = ps_small.tile([128, 16], F32, tag="ph")
                nc.tensor.matmul(
                    ph[:, :], w1_sb[:, hc * 128:(hc + 1) * 128], mlp_sb[:, :],
                    start=True, stop=True,
                )
                # relu(x + b1) = max(x + b1, 0) in one DVE op
                nc.vector.tensor_scalar(
                    h_sb[:, hc * 16:(hc + 1) * 16], ph[:, :],
                    bias[:, hc:hc + 1], 0.0,
                    mybir.AluOpType.add, mybir.AluOpType.max,
                )
            filler(fill_mlp)

            # ---- per tap: W assembly (MLP layer 2) + K=64 conv matmul ----
            for t in range(9):
                if t == 2:
                    filler(fill_b)
                if t == 6:
                    filler(fill_c)
                w_sb = wpool.tile([64, 48], BF16, tag="W")
                for o in range(3):
                    pw = ps_w.tile([64, 16], F32, tag="pw")
                    for hc in range(2):
                        nc.tensor.matmul(
                            pw[:, :], w2_slice(t, o, hc),
                            h_sb[:, hc * 16:(hc + 1) * 16],
                            start=(hc == 0), stop=(hc == 1),
                        )
                    nc.vector.tensor_scalar_add(
                        w_sb[:, o * 16:(o + 1) * 16], pw[:, :],
                        bias[0:64, 2 + t * 3 + o:3 + t * 3 + o],
                    )
                if t == 0:
                    filler(fill_band)
                off = (t // 3) * 66 + (t % 3)
                rhs = blob_band[0:64, off:off + 8 * 66].rearrange(
                    "p (r c) -> p r c", c=66
                )[:, :, 0:64]
                nc.tensor.matmul(
                    rgb_ps[:, :], w_sb[:, :], rhs,
                    start=(t == 0), stop=(t == 8),
                )

            # ---- write out (bf16, host upcasts) ----
            out_sb = opool.tile([48, NPOS], BF16, tag="out")
            nc.vector.tensor_copy(out_sb[:, :], rgb_ps[:, :])
            nc.scalar.dma_start(out48[:, :], out_sb[:, :])

    nc.compile()
    return nc


def _host_prep(feat, w1, b1, w2, b2):
    """Pack shared blobs + per-core band blobs (bf16)."""
    import ml_dtypes
    bf16 = ml_dtypes.bfloat16
    feat = np.ascontiguousarray(np.asarray(feat, dtype=np.float32))[0]  # [64,64,64]
    w1 = np.asarray(w1, dtype=np.float32)
    b1 = np.asarray(b1, dtype=np.float32)
    w2 = np.asarray(w2, dtype=np.float32)
    b2 = np.asarray(b2, dtype=np.float32)

    dydx = np.arange(16)
    mlpin = np.stack(
        [dydx // 4 / 4.0, dydx % 4 / 4.0, np.full(16, 0.25)], axis=0
    ).astype(np.float32)  # [3, 16]

    w2r = w2.reshape(256, 64, 9, 3).astype(bf16)  # [h, c, t, o]
    b2r = b2.reshape(64, 9, 3)                    # [c, t, o]

    bias = np.zeros((128, 29), dtype=np.float32)
    bias[:, 0] = b1[0:128]
    bias[:, 1] = b1[128:256]
    for t in range(9):
        for o in range(3):
            bias[0:64, 2 + t * 3 + o] = b2r[:, t, o]

    blob_w1x = np.zeros((128, COLS_W1X), dtype=bf16)
    blob_w1x[:, 0:58] = bias.view(bf16)
    blob_w1x[0:3, OFF_W1:OFF_W1 + 256] = w1.astype(bf16)
    blob_w1x[0:3, OFF_MLP:OFF_MLP + 16] = mlpin.astype(bf16)

    blob_a = np.empty((128, COLS_A), dtype=bf16)
    blob_b = np.empty((128, COLS_B), dtype=bf16)
    blob_c = np.empty((128, COLS_C), dtype=bf16)
    for t in range(9):
        dst, toff = (
            (blob_a, t) if t < 2 else (blob_b, t - 2) if t < 6 else (blob_c, t - 6)
        )
        for o in range(3):
            for hc in range(2):
                base = toff * 384 + (o * 2 + hc) * 64
                dst[:, base:base + 64] = w2r[hc * 128:(hc + 1) * 128, :, t, o]

    featp = np.zeros((64, 66, 66), dtype=bf16)
    featp[:, 1:65, 1:65] = feat.astype(bf16)

    blobs_band = []
    for core in range(N_CORES):
        r0 = core * ROWS_PER_CORE
        bb = np.zeros((64, COLS_BAND), dtype=bf16)
        bb[:, 0:660] = featp[:, r0:r0 + BAND_ROWS, :].reshape(64, BAND_ROWS * 66)
        blobs_band.append(bb)
    return blob_w1x, blob_a, blob_b, blob_c, blobs_band


def _assemble(per_core_out48):
    """[8 x [48, 512] bf16] -> [1, 3, 256, 256] f32."""
    full = np.stack([np.asarray(o).astype(np.float32) for o in per_core_out48])
    full = full.reshape(8, 3, 4, 4, 8, 64)               # [core, o, dy, dx, r, x]
    rgb = full.transpose(1, 0, 4, 2, 5, 3).reshape(3, 256, 256)
    return np.ascontiguousarray(rgb)[None]


def get_program():
    cfg = (WARM_BIG, WARM_SMALL, FILL_MLP, FILL_BAND, FILL_B, FILL_C)
    if cfg not in _CACHE:
        _CACHE[cfg] = _build_program(cfg)
    return _CACHE[cfg]


def run(feat, w1, b1, w2, b2, out_h, out_w, trace=False, **spmd_kwargs):
    assert int(out_h) == 256 and int(out_w) == 256
    nc = get_program()
    blob_w1x, blob_a, blob_b, blob_c, blobs_band = _host_prep(feat, w1, b1, w2, b2)
    in_maps = [
        {"blob_w1x": blob_w1x, "blob_a": blob_a, "blob_b": blob_b,
         "blob_c": blob_c, "blob_band": blobs_band[core]}
        for core in range(N_CORES)
    ]
    res = run_bass_kernel_spmd(
        nc, in_maps, core_ids=list(range(N_CORES)), trace=trace, **spmd_kwargs
    )
    out = _assemble([res.results[core]["out48"] for core in range(N_CORES)])
    return out, res


def kernel(feat, w1, b1, w2, b2, out_h, out_w):
    out, _ = run(feat, w1, b1, w2, b2, out_h, out_w, trace=False)
    return out


# revision 4
# speedup vs baseline: 1.0378x; 1.0378x over previous
"""MetaSR super-resolution Trainium2 kernel (bf16, paired-tap conv).

Structure exploited: out_h=out_w=256 with H=W=64 LR grid means the scale
factor is exactly 4, so the nearest-neighbor gather index is iy=oy//4,
ix=ox//4 and the per-query MLP input collapses to 16 distinct subpixel
phases [dy/4, dx/4, 0.25].  The whole model becomes:

  1. h    = relu(mlp_in @ w1 + b1)              [16, 256]
  2. predw = h @ w2 + b2                        [16, 576, 3]
  3. rgb[o, 4*iy+dy, 4*ix+dx] =
       sum_{c,ki,kj} feat[c, iy+ki-1, ix+kj-1] * predw[(dy,dx), c*9+ki*3+kj, o]
     i.e. a 3x3 conv with 64 in / 48 out channels + pixel shuffle.

Sharding: data-parallel over LR rows (8 rows per core, 10-row halo band),
weights replicated; steps 1+2 are recomputed on every core (tiny).

The conv contraction (K = 9 taps x 64 ch = 576) is chunked K=128 by pairing
taps: the zero-padded band lives twice in a 128-partition tile at free-dim
offsets differing by the two taps' shift delta, so one K=128 matmul
consumes two taps (512-column matmuls cost ~620ns flat, so fewer matmuls
wins over less DMA).  band free index = r*66 + x, tap (ki,kj) shift =
ki*66+kj; taps are paired with shift deltas 1 or 64.

All large operands are bf16 (host-side cast); verified end-to-end rel err
~4e-3 vs the f32 reference (budget 2e-2).

Scheduling: one DMA queue (Sync) carries everything in strict consumption
order (bias+w1 first at 78KB so the MLP unblocks immediately, then w2
chunk m0, the band, m1+m2, m3+m4, and finally the output store on the
same, already-warm queue).  Dummy "filler" matmuls -- given explicit data
dependencies on h / the latest W tile so the scheduler cannot hoist them
-- keep the PE busy across the known DMA wait points.
"""

import os

import numpy as np

try:
    import concourse.bass as bass
except ImportError:  # fall back to the repo checkout
    import sys
    sys.path.insert(0, "/opt/trn_rl_repo")
    import concourse.bass as bass
import concourse.mybir as mybir
import concourse.tile as tile
from concourse import bacc
from concourse.bass_utils import run_bass_kernel_spmd

F32 = mybir.dt.float32
BF16 = mybir.dt.bfloat16
N_CORES = 8
ROWS_PER_CORE = 8          # LR rows per core
BAND_ROWS = ROWS_PER_CORE + 2
NPOS = ROWS_PER_CORE * 64  # 512 LR positions per core

# Tap order for K-chunking.  Taps t = ki*3+kj have band shift ki*66+kj:
#   t:      0   1   2   3    4    5    6    7    8
#   shift:  0   1   2   66   67   68   132  133  134
# chunk0: [t0; t1] band1 off 1 | chunk1: [t3; t2] band2 off 66
# chunk2: [t4; t5] band1 off 68 | chunk3: [t6; t7] band1 off 133
# chunk4: [t8] band2 off 134 (K=64)
TAP_ORDER = [0, 1, 3, 2, 4, 5, 6, 7, 8]
CHUNK_SPECS = [  # (band_tile_idx, rhs_offset, K)
    (0, 1, 128),
    (1, 66, 128),
    (0, 68, 128),
    (0, 133, 128),
    (1, 134, 64),
]

# blob_w1x [128, 306] bf16: bias (17 f32 = 34 bf16) | w1 [3,256] | mlpin [3,16]
OFF_W1 = 34
OFF_MLP = 34 + 256
COLS_W1X = 306
# w2 m-chunk blocks: (o*2+hc) x [128, msize]
COLS_A = 768            # m0
COLS_B = 768 * 2        # m1, m2
COLS_C = 768 + 384      # m3, m4
# blob_band: band1 [128, 661] + band2 [128, 724]
OFF_BAND2 = 661
COLS_BAND = 1385


def _env(name, default):
    return int(os.environ.get(name, str(default)))

WARM_BIG = _env("METASR_WARM_BIG", 2)
WARM_SMALL = _env("METASR_WARM_SMALL", 2)
# filler counts: after mlp1, then after each chunk's W assembly
FILLS = [int(x) for x in os.environ.get("METASR_FILLS", "2,5,4,2,1,0").split(",")]

_CACHE = {}


def _build_program(cfg):
    """Build + compile the single-core Bass program (same for all cores)."""
    warm_big, warm_small, fills = cfg[0], cfg[1], list(cfg[2])
    nc = bacc.Bacc("TRN2", target_bir_lowering=False, debug=False)

    blob_w1x_d = nc.dram_tensor("blob_w1x", [128, COLS_W1X], BF16, kind="ExternalInput")
    blob_a_d = nc.dram_tensor("blob_a", [128, COLS_A], BF16, kind="ExternalInput")
    blob_band_d = nc.dram_tensor("blob_band", [128, COLS_BAND], BF16, kind="ExternalInput")
    blob_b_d = nc.dram_tensor("blob_b", [128, COLS_B], BF16, kind="ExternalInput")
    blob_c_d = nc.dram_tensor("blob_c", [128, COLS_C], BF16, kind="ExternalInput")
    out48 = nc.dram_tensor("out48", [48, NPOS], BF16, kind="ExternalOutput")

    with tile.TileContext(nc) as tc:
        with (
            tc.tile_pool(name="blobs", bufs=1) as blobs,
            tc.tile_pool(name="work", bufs=1) as work,
            tc.tile_pool(name="wpool", bufs=5) as wpool,
            tc.tile_pool(name="opool", bufs=1) as opool,
            tc.tile_pool(name="ps_small", bufs=2, space="PSUM") as ps_small,
            tc.tile_pool(name="ps_w", bufs=4, space="PSUM") as ps_w,
            tc.tile_pool(name="ps_warm", bufs=1, space="PSUM") as ps_warm,
            tc.tile_pool(name="ps_rgb", bufs=1, space="PSUM") as ps_rgb,
        ):
            # One queue (Sync), strict consumption order.
            blob_w1x = blobs.tile([128, COLS_W1X], BF16, tag="blob_w1x")
            nc.sync.dma_start(blob_w1x[:, :], blob_w1x_d[:, :])
            blob_a = blobs.tile([128, COLS_A], BF16, tag="blob_a")
            nc.sync.dma_start(blob_a[:, :], blob_a_d[:, :])
            blob_band = blobs.tile([128, COLS_BAND], BF16, tag="blob_band")
            nc.sync.dma_start(blob_band[:, :], blob_band_d[:, :])
            blob_b = blobs.tile([128, COLS_B], BF16, tag="blob_b")
            nc.sync.dma_start(blob_b[:, :], blob_b_d[:, :])
            blob_c = blobs.tile([128, COLS_C], BF16, tag="blob_c")
            nc.sync.dma_start(blob_c[:, :], blob_c_d[:, :])

            w1_sb = blob_w1x[0:3, OFF_W1:OFF_W1 + 256]
            mlp_sb = blob_w1x[0:3, OFF_MLP:OFF_MLP + 16]
            bias = blob_w1x.bitcast(F32)[:, 0:17]
            band_tiles = [
                blob_band[:, 0:661],
                blob_band[:, OFF_BAND2:OFF_BAND2 + 724],
            ]

            def w2_slice(m, o, hc, msize):
                col = (o * 2 + hc) * msize
                if m == 0:
                    return blob_a[:, col:col + msize]
                if m <= 2:
                    base = (m - 1) * 768 + col
                    return blob_b[:, base:base + msize]
                base = (m - 3) * 768 + col
                return blob_c[:, base:base + msize]

            # ---- PE warm-up / fillers: dummy zero matmuls keep the PE busy
            # (and its clock ramped) across DMA waits.  Fillers take a `dep`
            # operand so the tile scheduler cannot hoist them ahead of the
            # real work they are meant to follow.
            rgb_ps = ps_rgb.tile([48, NPOS], F32, tag="rgb")
            warm_ps = ps_warm.tile([128, 128], F32, tag="warm_ps")
            warm = work.tile([128, 512], BF16, tag="warm")
            nc.vector.memset(warm[:, :], 0.0)

            def filler(n, dep=None):
                for _ in range(n):
                    if dep is None:
                        nc.tensor.matmul(
                            warm_ps[:, :], warm[:, 0:128], warm[:, 0:128],
                            start=True, stop=True,
                        )
                    else:
                        k, mcols = dep.shape
                        nc.tensor.matmul(
                            warm_ps[0:mcols, 0:128], dep, warm[0:k, 0:128],
                            start=True, stop=True,
                        )

            for _ in range(warm_big):
                nc.tensor.matmul(
                    rgb_ps[:, :], warm[:, 0:48], warm[:, 0:NPOS],
                    start=True, stop=True,
                )
            filler(warm_small)

            # ---- MLP layer 1: h_actT [256, 16] in two 128-chunks ----
            h_sb = work.tile([128, 32], BF16, tag="hact")
            for hc in range(2):
                ph = ps_small.tile([128, 16], F32, tag="ph")
                nc.tensor.matmul(
                    ph[:, :], w1_sb[:, hc * 128:(hc + 1) * 128], mlp_sb[:, :],
                    start=True, stop=True,
                )
                # relu(x + b1) = max(x + b1, 0) in one DVE op
                nc.vector.tensor_scalar(
                    h_sb[:, hc * 16:(hc + 1) * 16], ph[:, :],
                    bias[:, hc:hc + 1], 0.0,
                    mybir.AluOpType.add, mybir.AluOpType.max,
                )
            filler(fills[0], dep=h_sb[:, 0:32])

            # ---- per K-chunk: W assembly (MLP layer 2) + conv matmul ----
            for m, (bidx, roff, K) in enumerate(CHUNK_SPECS):
                msize = K
                w_sb = wpool.tile([128, 48], BF16, tag="W")
                for o in range(3):
                    pw = ps_w.tile([128, 16], F32, tag="pw")
                    for hc in range(2):
                        nc.tensor.matmul(
                            pw[:msize, :],
                            w2_slice(m, o, hc, msize),
                            h_sb[:, hc * 16:(hc + 1) * 16],
                            start=(hc == 0), stop=(hc == 1),
                        )
                    nc.vector.tensor_scalar_add(
                        w_sb[:msize, o * 16:(o + 1) * 16], pw[:msize, :],
                        bias[:msize, 2 + o * 5 + m:3 + o * 5 + m],
                    )
                filler(fills[1 + m], dep=w_sb[0:msize, 0:48])
                bt = band_tiles[bidx]
                rhs = bt[0:K, roff:roff + 8 * 66].rearrange(
                    "p (r c) -> p r c", c=66
                )[:, :, 0:64]
                nc.tensor.matmul(
                    rgb_ps[:, :], w_sb[:msize, :], rhs,
                    start=(m == 0), stop=(m == len(CHUNK_SPECS) - 1),
                )

            # ---- write out (bf16, host upcasts) on the warm Sync queue ----
            out_sb = opool.tile([48, NPOS], BF16, tag="out")
            nc.vector.tensor_copy(out_sb[:, :], rgb_ps[:, :])
            nc.sync.dma_start(out48[:, :], out_sb[:, :])

    nc.compile()
    return nc


def _host_prep(feat, w1, b1, w2, b2):
    """Pack shared blobs + per-core band blobs (bf16)."""
    import ml_dtypes
    bf16 = ml_dtypes.bfloat16
    feat = np.ascontiguousarray(np.asarray(feat, dtype=np.float32))[0]  # [64,64,64]
    w1 = np.asarray(w1, dtype=np.float32)
    b1 = np.asarray(b1, dtype=np.float32)
    w2 = np.asarray(w2, dtype=np.float32)
    b2 = np.asarray(b2, dtype=np.float32)

    dydx = np.arange(16)
    mlpin = np.stack(
        [dydx // 4 / 4.0, dydx % 4 / 4.0, np.full(16, 0.25)], axis=0
    ).astype(np.float32)  # [3, 16]

    # tap-major permutations of w2/b2
    w2r = w2.reshape(256, 64, 9, 3)  # [h, c, t, o]
    w2p = np.empty((3, 256, 576), dtype=np.float32)
    b2r = b2.reshape(64, 9, 3)       # [c, t, o]
    b2p = np.empty((3, 576), dtype=np.float32)
    for blk, t in enumerate(TAP_ORDER):
        w2p[:, :, blk * 64:(blk + 1) * 64] = w2r[:, :, t, :].transpose(2, 0, 1)
        b2p[:, blk * 64:(blk + 1) * 64] = b2r[:, t, :].T
    w2p = w2p.astype(bf16)

    bias = np.zeros((128, 17), dtype=np.float32)
    bias[:, 0] = b1[0:128]
    bias[:, 1] = b1[128:256]
    for o in range(3):
        for m in range(5):
            msize = 128 if m < 4 else 64
            bias[:msize, 2 + o * 5 + m] = b2p[o, 128 * m:128 * m + msize]

    blob_w1x = np.zeros((128, COLS_W1X), dtype=bf16)
    blob_w1x[:, 0:34] = bias.view(bf16)
    blob_w1x[0:3, OFF_W1:OFF_W1 + 256] = w1.astype(bf16)
    blob_w1x[0:3, OFF_MLP:OFF_MLP + 16] = mlpin.astype(bf16)

    blob_a = np.empty((128, COLS_A), dtype=bf16)
    blob_b = np.empty((128, COLS_B), dtype=bf16)
    blob_c = np.empty((128, COLS_C), dtype=bf16)
    for m in range(5):
        msize = 128 if m < 4 else 64
        dst, moff = (
            (blob_a, 0) if m == 0 else
            (blob_b, (m - 1) * 768) if m <= 2 else
            (blob_c, (m - 3) * 768)
        )
        for o in range(3):
            for hc in range(2):
                base = moff + (o * 2 + hc) * msize
                dst[:, base:base + msize] = \
                    w2p[o, hc * 128:(hc + 1) * 128, 128 * m:128 * m + msize]

    featp = np.zeros((64, 66, 66), dtype=bf16)
    featp[:, 1:65, 1:65] = feat.astype(bf16)

    blobs_band = []
    for core in range(N_CORES):
        r0 = core * ROWS_PER_CORE
        band = featp[:, r0:r0 + BAND_ROWS, :].reshape(64, BAND_ROWS * 66)
        bb = np.zeros((128, COLS_BAND), dtype=bf16)
        bb[0:64, 1:661] = band
        bb[64:128, 0:660] = band
        bb[0:64, OFF_BAND2:OFF_BAND2 + 660] = band
        bb[64:128, OFF_BAND2 + 64:OFF_BAND2 + 724] = band
        blobs_band.append(bb)
    return blob_w1x, blob_a, blob_b, blob_c, blobs_band


def _assemble(per_core_out48):
    """[8 x [48, 512] bf16] -> [1, 3, 256, 256] f32."""
    full = np.stack([np.asarray(o).astype(np.float32) for o in per_core_out48])
    full = full.reshape(8, 3, 4, 4, 8, 64)               # [core, o, dy, dx, r, x]
    rgb = full.transpose(1, 0, 4, 2, 5, 3).reshape(3, 256, 256)
    return np.ascontiguousarray(rgb)[None]


def get_program():
    cfg = (WARM_BIG, WARM_SMALL, tuple(FILLS))
    if cfg not in _CACHE:
        _CACHE[cfg] = _build_program(cfg)
    return _CACHE[cfg]


def run(feat, w1, b1, w2, b2, out_h, out_w, trace=False, **spmd_kwargs):
    assert int(out_h) == 256 and int(out_w) == 256
    nc = get_program()
    blob_w1x, blob_a, blob_b, blob_c, blobs_band = _host_prep(feat, w1, b1, w2, b2)
    in_maps = [
        {"blob_w1x": blob_w1x, "blob_a": blob_a, "blob_b": blob_b,
         "blob_c": blob_c, "blob_band": blobs_band[core]}
        for core in range(N_CORES)
    ]
    res = run_bass_kernel_spmd(
        nc, in_maps, core_ids=list(range(N_CORES)), trace=trace, **spmd_kwargs
    )
    out = _assemble([res.results[core]["out48"] for core in range(N_CORES)])
    return out, res


def kernel(feat, w1, b1, w2, b2, out_h, out_w):
    out, _ = run(feat, w1, b1, w2, b2, out_h, out_w, trace=False)
    return out


# revision 8
# speedup vs baseline: 1.0681x; 1.0292x over previous
"""MetaSR super-resolution Trainium2 kernel (bf16, hybrid paired/single taps).

Structure exploited: out_h=out_w=256 with H=W=64 LR grid means the scale
factor is exactly 4, so the nearest-neighbor gather index is iy=oy//4,
ix=ox//4 and the per-query MLP input collapses to 16 distinct subpixel
phases [dy/4, dx/4, 0.25].  The whole model becomes:

  1. h    = relu(mlp_in @ w1 + b1)              [16, 256]
  2. predw = h @ w2 + b2                        [16, 576, 3]
  3. rgb[o, 4*iy+dy, 4*ix+dx] =
       sum_{c,ki,kj} feat[c, iy+ki-1, ix+kj-1] * predw[(dy,dx), c*9+ki*3+kj, o]
     i.e. a 3x3 conv with 64 in / 48 out channels + pixel shuffle.

Sharding: data-parallel over LR rows (8 rows per core, 10-row halo band),
weights replicated; steps 1+2 are recomputed on every core (tiny).

Conv decomposition: one [128, 661] band tile holds the zero-padded band
twice -- partitions 0-63 at free-offset 1, partitions 64-127 at offset 0.
That single tile supports BOTH K=128 tap pairs with shift delta 1
([t0;t1], [t4;t5], [t6;t7]) AND any tap alone as a K=64 matmul from the
offset-0 half.  The remaining taps t2/t3/t8 run singly, for 6 conv
matmuls total off one 169KB band transfer.  (Measured: 512-column
matmuls cost ~620ns flat regardless of K, and the DMA engine pool
sustains only ~215GB/s however many queues are used -- so conv count and
total bytes are the two levers, and this layout is their best trade.)

All large operands are bf16 (host-side cast); verified end-to-end rel err
~4e-3 vs the f32 reference (budget 2e-2).

Scheduling: one DMA queue (Sync) carries everything in strict consumption
order (bias+w1 first at 78KB so the MLP unblocks immediately; per-chunk w2
blobs with the band after the first; the output store last on the same,
already-warm queue).  Dummy "filler" matmuls -- given explicit data
dependencies on h / the latest W tile so the tile scheduler cannot hoist
them -- keep the PE busy across the known DMA wait points.
"""

import os

import numpy as np

try:
    import concourse.bass as bass
except ImportError:  # fall back to the repo checkout
    import sys
    sys.path.insert(0, "/opt/trn_rl_repo")
    import concourse.bass as bass
import concourse.mybir as mybir
import concourse.tile as tile
from concourse import bacc
from concourse.bass_utils import run_bass_kernel_spmd

F32 = mybir.dt.float32
BF16 = mybir.dt.bfloat16
N_CORES = 8
ROWS_PER_CORE = 8          # LR rows per core
BAND_ROWS = ROWS_PER_CORE + 2
NPOS = ROWS_PER_CORE * 64  # 512 LR positions per core

# Taps t = ki*3+kj have band shift ki*66+kj:
#   t:      0   1   2   3    4    5    6    7    8
#   shift:  0   1   2   66   67   68   132  133  134
# band1 tile: partitions 0-63 = band @ col offset 1, partitions 64-127 =
# band @ offset 0.  Pair [ta; tb] with shift(tb) = shift(ta)+1 reads the
# full 128 partitions at rhs offset shift(tb); single tap t reads
# partitions 0-63 at rhs offset shift(t)+1 (matmul operands must share a
# base partition, so singles use the offset-1 half).
# chunks: ([taps], part_lo, rhs_offset, K)
CHUNK_SPECS = [
    ([0, 1], 0, 1, 128),
    ([4, 5], 0, 68, 128),
    ([6, 7], 0, 133, 128),
    ([2], 0, 3, 64),
    ([3], 0, 67, 64),
    ([8], 0, 135, 64),
]
N_CHUNKS = len(CHUNK_SPECS)

# blob_w1x [128, 306] bf16: bias (20 f32 = 40 bf16) | w1 [3,256] | mlpin [3,16]
OFF_W1 = 40
OFF_MLP = 40 + 256
COLS_W1X = 312
# per-chunk w2 blobs: (o*2+hc) x [128, msize]; band1 [128, 663]
# (663 = 134 + 8*66 + pad so the widest single-tap rhs slice stays in-bounds)
COLS_BAND = 663


def _env(name, default):
    return int(os.environ.get(name, str(default)))

WARM_BIG = _env("METASR_WARM_BIG", 2)
WARM_SMALL = _env("METASR_WARM_SMALL", 2)
# filler counts: after mlp1, then after each chunk's W assembly
FILLS = [int(x) for x in os.environ.get("METASR_FILLS", "5,4,2,2,1,1,0").split(",")]

_CACHE = {}


def _build_program(cfg):
    """Build + compile the single-core Bass program (same for all cores)."""
    warm_big, warm_small, fills = cfg[0], cfg[1], list(cfg[2])
    nc = bacc.Bacc("TRN2", target_bir_lowering=False, debug=False)

    blob_w1x_d = nc.dram_tensor("blob_w1x", [128, COLS_W1X], BF16, kind="ExternalInput")
    w2_d = [
        nc.dram_tensor(f"blob_w2_{c}", [128, 6 * spec[3]], BF16, kind="ExternalInput")
        for c, spec in enumerate(CHUNK_SPECS)
    ]
    blob_band_d = nc.dram_tensor("blob_band", [128, COLS_BAND], BF16, kind="ExternalInput")
    out48 = nc.dram_tensor("out48", [48, NPOS], BF16, kind="ExternalOutput")

    with tile.TileContext(nc) as tc:
        with (
            tc.tile_pool(name="blobs", bufs=1) as blobs,
            tc.tile_pool(name="work", bufs=1) as work,
            tc.tile_pool(name="wpool", bufs=6) as wpool,
            tc.tile_pool(name="opool", bufs=1) as opool,
            tc.tile_pool(name="ps_small", bufs=2, space="PSUM") as ps_small,
            tc.tile_pool(name="ps_w", bufs=4, space="PSUM") as ps_w,
            tc.tile_pool(name="ps_warm", bufs=1, space="PSUM") as ps_warm,
            tc.tile_pool(name="ps_rgb", bufs=1, space="PSUM") as ps_rgb,
        ):
            # One queue (Sync), strict consumption order: w1x, w2 chunk 0,
            # band, w2 chunks 1..5; output store joins the same queue last.
            blob_w1x = blobs.tile([128, COLS_W1X], BF16, tag="blob_w1x")
            nc.sync.dma_start(blob_w1x[:, :], blob_w1x_d[:, :])
            w2_sb = []
            blob_band = None
            for c, spec in enumerate(CHUNK_SPECS):
                t = blobs.tile([128, 6 * spec[3]], BF16, tag=f"w2_{c}")
                nc.sync.dma_start(t[:, :], w2_d[c][:, :])
                w2_sb.append(t)
                if c == 0:
                    blob_band = blobs.tile([128, COLS_BAND], BF16, tag="band")
                    nc.sync.dma_start(blob_band[:, :], blob_band_d[:, :])

            w1_sb = blob_w1x[0:3, OFF_W1:OFF_W1 + 256]
            mlp_sb = blob_w1x[0:3, OFF_MLP:OFF_MLP + 16]
            bias = blob_w1x.bitcast(F32)[:, 0:20]

            # ---- PE warm-up / fillers: dummy zero matmuls keep the PE busy
            # across DMA waits.  Fillers take a `dep` operand so the tile
            # scheduler cannot hoist them ahead of the real work they follow.
            # warm memset runs on gpsimd (idle) so warmups start early.
            rgb_ps = ps_rgb.tile([48, NPOS], F32, tag="rgb")
            warm_ps = ps_warm.tile([128, 128], F32, tag="warm_ps")
            warm = work.tile([128, 512], BF16, tag="warm")
            nc.gpsimd.memset(warm[:, :], 0.0)

            def filler(n, dep=None):
                for _ in range(n):
                    if dep is None:
                        nc.tensor.matmul(
                            warm_ps[:, :], warm[:, 0:128], warm[:, 0:128],
                            start=True, stop=True,
                        )
                    else:
                        k, mcols = dep.shape
                        nc.tensor.matmul(
                            warm_ps[0:mcols, 0:128], dep, warm[0:k, 0:128],
                            start=True, stop=True,
                        )

            for _ in range(warm_big):
                nc.tensor.matmul(
                    rgb_ps[:, :], warm[:, 0:48], warm[:, 0:NPOS],
                    start=True, stop=True,
                )
            filler(warm_small)

            # ---- MLP layer 1: h_actT [256, 16] in two 128-chunks ----
            h_sb = work.tile([128, 32], BF16, tag="hact")
            for hc in range(2):
                ph = ps_small.tile([128, 16], F32, tag="ph")
                nc.tensor.matmul(
                    ph[:, :], w1_sb[:, hc * 128:(hc + 1) * 128], mlp_sb[:, :],
                    start=True, stop=True,
                )
                # relu(x + b1) = max(x + b1, 0) in one DVE op
                nc.vector.tensor_scalar(
                    h_sb[:, hc * 16:(hc + 1) * 16], ph[:, :],
                    bias[:, hc:hc + 1], 0.0,
                    mybir.AluOpType.add, mybir.AluOpType.max,
                )
            filler(fills[0], dep=h_sb[:, 0:32])

            # ---- per chunk: W assembly (MLP layer 2) + conv matmul ----
            for c, (taps, plo, roff, K) in enumerate(CHUNK_SPECS):
                msize = K
                w_sb = wpool.tile([128, 48], BF16, tag="W")
                for o in range(3):
                    pw = ps_w.tile([128, 16], F32, tag="pw")
                    for hc in range(2):
                        nc.tensor.matmul(
                            pw[:msize, :],
                            w2_sb[c][:, (o * 2 + hc) * msize:(o * 2 + hc + 1) * msize],
                            h_sb[:, hc * 16:(hc + 1) * 16],
                            start=(hc == 0), stop=(hc == 1),
                        )
                    nc.vector.tensor_scalar_add(
                        w_sb[:msize, o * 16:(o + 1) * 16], pw[:msize, :],
                        bias[:msize, 2 + c * 3 + o:3 + c * 3 + o],
                    )
                filler(fills[1 + c], dep=w_sb[0:msize, 0:48])
                rhs = blob_band[plo:plo + K, roff:roff + 8 * 66].rearrange(
                    "p (r c) -> p r c", c=66
                )[:, :, 0:64]
                nc.tensor.matmul(
                    rgb_ps[:, :], w_sb[:msize, :], rhs,
                    start=(c == 0), stop=(c == N_CHUNKS - 1),
                )

            # ---- write out (bf16, host upcasts) on the warm Sync queue ----
            out_sb = opool.tile([48, NPOS], BF16, tag="out")
            nc.vector.tensor_copy(out_sb[:, :], rgb_ps[:, :])
            nc.sync.dma_start(out48[:, :], out_sb[:, :])

    nc.compile()
    return nc


def _host_prep(feat, w1, b1, w2, b2):
    """Pack shared blobs + per-core band blobs (bf16)."""
    import ml_dtypes
    bf16 = ml_dtypes.bfloat16
    feat = np.ascontiguousarray(np.asarray(feat, dtype=np.float32))[0]  # [64,64,64]
    w1 = np.asarray(w1, dtype=np.float32)
    b1 = np.asarray(b1, dtype=np.float32)
    w2 = np.asarray(w2, dtype=np.float32)
    b2 = np.asarray(b2, dtype=np.float32)

    dydx = np.arange(16)
    mlpin = np.stack(
        [dydx // 4 / 4.0, dydx % 4 / 4.0, np.full(16, 0.25)], axis=0
    ).astype(np.float32)  # [3, 16]

    w2r = w2.reshape(256, 64, 9, 3).astype(bf16)  # [h, c, t, o]
    b2r = b2.reshape(64, 9, 3)                    # [c, t, o]

    bias = np.zeros((128, 20), dtype=np.float32)
    bias[:, 0] = b1[0:128]
    bias[:, 1] = b1[128:256]
    for c, (taps, plo, roff, K) in enumerate(CHUNK_SPECS):
        for o in range(3):
            col = np.concatenate([b2r[:, t, o] for t in taps])
            bias[:K, 2 + c * 3 + o] = col

    blob_w1x = np.zeros((128, COLS_W1X), dtype=bf16)
    blob_w1x[:, 0:40] = bias.view(bf16)
    blob_w1x[0:3, OFF_W1:OFF_W1 + 256] = w1.astype(bf16)
    blob_w1x[0:3, OFF_MLP:OFF_MLP + 16] = mlpin.astype(bf16)

    w2_blobs = []
    for c, (taps, plo, roff, K) in enumerate(CHUNK_SPECS):
        blob = np.empty((128, 6 * K), dtype=bf16)
        for o in range(3):
            for hc in range(2):
                # [128 h, K ct] with ct = concatenated tap channel blocks
                block = np.concatenate(
                    [w2r[hc * 128:(hc + 1) * 128, :, t, o] for t in taps], axis=1
                )
                blob[:, (o * 2 + hc) * K:(o * 2 + hc + 1) * K] = block
        w2_blobs.append(blob)

    featp = np.zeros((64, 66, 66), dtype=bf16)
    featp[:, 1:65, 1:65] = feat.astype(bf16)

    blobs_band = []
    for core in range(N_CORES):
        r0 = core * ROWS_PER_CORE
        band = featp[:, r0:r0 + BAND_ROWS, :].reshape(64, BAND_ROWS * 66)
        bb = np.zeros((128, COLS_BAND), dtype=bf16)
        bb[0:64, 1:661] = band
        bb[64:128, 0:660] = band
        blobs_band.append(bb)
    return blob_w1x, w2_blobs, blobs_band


def _assemble(per_core_out48):
    """[8 x [48, 512] bf16] -> [1, 3, 256, 256] f32."""
    full = np.stack([np.asarray(o).astype(np.float32) for o in per_core_out48])
    full = full.reshape(8, 3, 4, 4, 8, 64)               # [core, o, dy, dx, r, x]
    rgb = full.transpose(1, 0, 4, 2, 5, 3).reshape(3, 256, 256)
    return np.ascontiguousarray(rgb)[None]


def get_program():
    cfg = (WARM_BIG, WARM_SMALL, tuple(FILLS))
    if cfg not in _CACHE:
        _CACHE[cfg] = _build_program(cfg)
    return _CACHE[cfg]


def run(feat, w1, b1, w2, b2, out_h, out_w, trace=False, **spmd_kwargs):
    assert int(out_h) == 256 and int(out_w) == 256
    nc = get_program()
    blob_w1x, w2_blobs, blobs_band = _host_prep(feat, w1, b1, w2, b2)
    in_maps = []
    for core in range(N_CORES):
        m = {"blob_w1x": blob_w1x, "blob_band": blobs_band[core]}
        for c, blob in enumerate(w2_blobs):
            m[f"blob_w2_{c}"] = blob
        in_maps.append(m)
    res = run_bass_kernel_spmd(
        nc, in_maps, core_ids=list(range(N_CORES)), trace=trace, **spmd_kwargs
    )
    out = _assemble([res.results[core]["out48"] for core in range(N_CORES)])
    return out, res


def kernel(feat, w1, b1, w2, b2, out_h, out_w):
    out, _ = run(feat, w1, b1, w2, b2, out_h, out_w, trace=False)
    return out
